# revision 27
# baseline (speedup 1.0000x reference)
"""Multi-head attention (B=4, S=2048, E=1024, H=16, D=64) on 8 Trainium2 cores.

Sharding: core c handles batch b=c//2 and head-group g=c%2 (8 of the 16 heads).
Each core computes, for its (batch, head-group):
  qk_T = (x_b @ w_qk + b_qk)^T        transposed QK projection  [1024, S]
  V    = x_b @ w_v + b_v              natural V projection      [S, 512]
  per head: S_T = K @ Q^T, P^T = exp(S_T/8),
            attn' = V'^T @ P^T   (V' = [1|1|0pad|V], so attn'[0] holds the
                                  softmax denominators, attn'[64:128] = P V)
            attn_T = attn'[64:128] * recip(attn'[0]) partition-broadcast
  out_T partial = (attn_T^T @ w_out_rows)^T                     [1024, S]
Host sums the two head-group partials per batch, transposes, adds b_out.

Default variant v5 (build_nc_v4 + bf16): bf16 operands everywhere (FWL weight
loads, halved DMA), q-block phase C with double-buffered score PSUM, the QK
projections of head-pair u+1 interleaved into pair u's attention (2 spare PSUM
banks), gpsimd partition_broadcast softmax normalization, bf16 output partials.
HW pitfalls encoded here: matmul weights need an even column count in bf16,
gpsimd/custom-DVE ops ignore AP partition offsets (inputs must sit at
partition 0), 64-partition DVE accesses must start at partition 0 or 64.
"""
import os
import sys

sys.path.insert(0, "/opt/trn_rl_repo")

import numpy as np

import concourse.bass as bass
import concourse.mybir as mybir
import concourse.tile as tile
from concourse import bacc
from concourse.bass_utils import run_bass_kernel_spmd

B, S, E, H, D = 4, 2048, 1024, 16, 64
HPC = 8            # heads per core
NCORES = 8
P = 128
f32 = mybir.dt.float32
f32r = mybir.dt.float32r
AF = mybir.ActivationFunctionType
SCALE = 1.0 / 8.0  # 1/sqrt(D)

# module-level stash so test.py can reuse the compiled kernel / results
_BUILD_CACHE = {}
LAST_RESULTS = None


def build_nc(s=S, repeat=1, half_scores=False):
    """Build + compile the per-core Bass program. Same NEFF for all 8 cores."""
    nsq = s // 512        # 512-wide s chunks
    nst = s // P          # 128-wide s tiles
    qk_dt = f32r
    nhalf = 2 if (half_scores and nsq >= 2) else 1
    cw = s // nhalf       # score/exp chunk width
    nc = bacc.Bacc("TRN2", target_bir_lowering=False, debug=False,
                   num_devices=NCORES)

    xT = nc.dram_tensor("xT", [E, s], f32r, kind="ExternalInput").ap()
    w_qk = nc.dram_tensor("w_qk", [E, HPC * 128], f32r, kind="ExternalInput").ap()
    b_qk = nc.dram_tensor("b_qk", [HPC * 128, 1], f32, kind="ExternalInput").ap()
    w_v = nc.dram_tensor("w_v", [E, HPC * D], f32r, kind="ExternalInput").ap()
    b_v = nc.dram_tensor("b_v", [P, HPC * D], f32, kind="ExternalInput").ap()
    w_out = nc.dram_tensor("w_out", [HPC * D, E], f32r, kind="ExternalInput").ap()
    outT = nc.dram_tensor("outT", [E, s], f32, kind="ExternalOutput").ap()
    scratch = nc.dram_tensor("scratch", [HPC, s], f32).ap()  # denominators bounce

    xT_r = xT.rearrange("(ko p) s -> p ko s", p=P)        # [128, 8, s]
    wqk_r = w_qk.rearrange("(ko p) f -> p ko f", p=P)     # [128, 8, 1024]
    wv_r = w_v.rearrange("(ko p) f -> p ko f", p=P)       # [128, 8, 512]
    bqk_r = b_qk.rearrange("(m p) one -> p (m one)", p=P)  # [128, 8]
    wo_r = w_out.rearrange("(j p) f -> p j f", p=P)       # [128, 4, 1024]
    outT_r = outT.rearrange("(m p) s -> p m s", p=P)      # [128, 8, s]

    with tile.TileContext(nc) as tc:
        def body():
            from contextlib import ExitStack
            with ExitStack() as outer:
                persist = outer.enter_context(tc.tile_pool(name="persist", bufs=1))
                qT2 = persist.tile([P, HPC // 2, s], qk_dt)  # [64*2 packed, pair, s]
                kT2 = persist.tile([P, HPC // 2, s], qk_dt)
                v_sb = persist.tile([P, nst, HPC, D + 1], f32r)  # V' with ones col
                bqk_sb = persist.tile([P, HPC], f32)
                bv_sb = persist.tile([P, HPC, D], f32)
                nc.sync.dma_start(bqk_sb[:], bqk_r)
                nc.sync.dma_start(bv_sb[:], b_v.rearrange("p (h d) -> p h d", d=D))
                for st in range(nst):
                    nc.vector.memset(v_sb[:, st, :, D:D + 1].bitcast(f32), 1.0)

                # ---- Phase A/B: projections, streaming x^T in 512-col chunks
                with ExitStack() as ab:
                    xpool = ab.enter_context(tc.tile_pool(name="x", bufs=2))
                    wqk_pool = ab.enter_context(tc.tile_pool(name="wqk", bufs=1))
                    wv_pool = ab.enter_context(tc.tile_pool(name="wv", bufs=1))
                    psA = ab.enter_context(
                        tc.tile_pool(name="psA", bufs=3, space="PSUM"))
                    psB = ab.enter_context(
                        tc.tile_pool(name="psB", bufs=2, space="PSUM"))
                    wqk_sb = wqk_pool.tile([P, 8, HPC * 128], f32r)
                    wv_sb = wv_pool.tile([P, 8, HPC * D], f32r)
                    nc.sync.dma_start(wqk_sb[:], wqk_r)
                    nc.sync.dma_start(wv_sb[:], wv_r)

                    for q in range(nsq):
                        sq = slice(q * 512, (q + 1) * 512)
                        xt = xpool.tile([P, 8, 512], f32r)
                        nc.sync.dma_start(xt[:], xT_r[:, :, sq])
                        for m in range(HPC):  # qk feature tiles of 128
                            ps = psA.tile([P, 512], f32)
                            for k in range(8):
                                nc.tensor.matmul(
                                    ps[:], lhsT=wqk_sb[:, k, m * P:(m + 1) * P],
                                    rhs=xt[:, k, :],
                                    start=(k == 0), stop=(k == 7))
                            dst = qT2 if m % 2 == 0 else kT2
                            nc.vector.tensor_scalar_add(
                                dst[:, m // 2, sq], ps[:], bqk_sb[:, m:m + 1])
                        for stl in range(4):  # s tiles of 128 in this chunk
                            st = q * 4 + stl
                            ps = psB.tile([P, 512], f32)
                            for k in range(8):
                                nc.tensor.matmul(
                                    ps[:], lhsT=xt[:, k, stl * P:(stl + 1) * P],
                                    rhs=wv_sb[:, k, :],
                                    start=(k == 0), stop=(k == 7))
                            nc.vector.tensor_add(
                                v_sb[:, st, :, 0:D],
                                ps.rearrange("p (h d) -> p h d", d=D),
                                bv_sb[:])

                # ---- Phases C+D share the attnT pool (opened after A/B frees
                # x/w space)
                cd = outer.enter_context(ExitStack())
                attnT_pool = cd.enter_context(tc.tile_pool(name="attnT", bufs=1))
                attnT = attnT_pool.tile([P, HPC * D // P, s], f32r)

                # ---- Phase C: attention per head
                with ExitStack() as c:
                    psS = c.enter_context(
                        tc.tile_pool(name="psS", bufs=nhalf, space="PSUM"))
                    psAt = c.enter_context(
                        tc.tile_pool(name="psAt", bufs=4, space="PSUM"))
                    ppool = c.enter_context(tc.tile_pool(name="pT", bufs=3))
                    npool = c.enter_context(tc.tile_pool(name="norm", bufs=2))
                    spool = c.enter_context(tc.tile_pool(name="asb", bufs=2))
                    for i in range(HPC):
                        u, poff = i // 2, (i % 2) * 64
                        QT = qT2[poff:poff + 64, u, :]
                        KT = kT2[poff:poff + 64, u, :]
                        at_tiles = [psAt.tile([D + 1, 512], f32,
                                              name=f"at{q}", tag=f"at{q}",
                                              bufs=1)
                                    for q in range(nsq)]
                        ncq = cw // 512  # 512-chunks per exp group

                        def emit_pv(skt, h, pT):
                            for q in range(ncq):
                                qq = h * ncq + q
                                nc.tensor.matmul(
                                    at_tiles[qq][:],
                                    lhsT=v_sb[:, skt, i, :],
                                    rhs=pT[:, q * 512:(q + 1) * 512],
                                    start=(skt == 0),
                                    stop=(skt == nst - 1))

                        prev = None  # defer PV one step: scores(k+1) precede
                        for skt in range(nst):
                            for h in range(nhalf):
                                ps_s = psS.tile([P, cw], f32,
                                                name="ps_s", tag="ps_s")
                                for q in range(ncq):
                                    qq = h * ncq + q
                                    nc.tensor.matmul(
                                        ps_s[:, q * 512:(q + 1) * 512],
                                        lhsT=KT[:, skt * P:(skt + 1) * P],
                                        rhs=QT[:, qq * 512:(qq + 1) * 512],
                                        start=True, stop=True)
                                pT = ppool.tile([P, cw], f32r,
                                                name="pT", tag="pT")
                                nc.scalar.activation(pT[:], ps_s[:], AF.Exp,
                                                     scale=SCALE)
                                if prev is not None:
                                    emit_pv(*prev)
                                prev = (skt, h, pT)
                        emit_pv(*prev)
                        # evacuate attn accumulators to SBUF (frees PSUM banks
                        # so the next head's PV can proceed during this norm)
                        attn_sb = spool.tile([D + 1, s], f32)
                        for q in range(nsq):
                            nc.vector.tensor_copy(
                                attn_sb[:, q * 512:(q + 1) * 512],
                                at_tiles[q][:])
                        # softmax normalization: recip of denominators (row 64),
                        # broadcast over 64 partitions via DRAM bounce
                        recip = npool.tile([1, s], f32, tag="recip")
                        nc.vector.reciprocal(recip[:], attn_sb[D:D + 1, :])
                        nc.sync.dma_start(scratch[i:i + 1, :], recip[:])
                        bc = npool.tile([64, s], f32, tag="bc")
                        nc.sync.dma_start(
                            bc[:], scratch[i:i + 1, :].partition_broadcast(64)
                            .rearrange("p one s -> p (one s)"))
                        nc.vector.tensor_mul(
                            attnT[poff:poff + 64, i // 2, :],
                            attn_sb[0:D, :], bc[:])

                # ---- Phase D: output projection (partial; host sums pairs)
                with ExitStack() as d:
                    wo_pool = d.enter_context(tc.tile_pool(name="wo", bufs=1))
                    psD = d.enter_context(
                        tc.tile_pool(name="psD", bufs=2, space="PSUM"))
                    opool = d.enter_context(tc.tile_pool(name="osb", bufs=2))
                    wo_sb = wo_pool.tile([P, 4, E], f32r)
                    nc.sync.dma_start(wo_sb[:], wo_r)
                    for m in range(8):
                        ps_o = psD.tile([P, s], f32)
                        for k in range(4):
                            for n4 in range(nsq):
                                nc.tensor.matmul(
                                    ps_o[:, n4 * 512:(n4 + 1) * 512],
                                    lhsT=wo_sb[:, k, m * P:(m + 1) * P],
                                    rhs=attnT[:, k, n4 * 512:(n4 + 1) * 512],
                                    start=(k == 0), stop=(k == 3))
                        o_sb = opool.tile([P, s], f32)
                        nc.vector.tensor_copy(o_sb[:], ps_o[:])
                        nc.sync.dma_start(outT_r[:, m, :], o_sb[:])

        if repeat > 1:
            with tc.For_i(0, repeat, 1):
                body()
        else:
            body()

    nc.compile()
    return nc


bf16 = mybir.dt.bfloat16


def build_nc_v2(s=S, repeat=1, qk_dt=f32r):
    """v2: q-block attention (2 blocks of 1024 queries/head), double-buffered
    score PSUM + double-buffered 2-bank PV accumulators, bf16 for pT/V/attnT/
    w_out, gpsimd partition_broadcast for the softmax denominator (no DRAM
    bounce), fast reciprocal. qk_dt=bf16 (v3) also puts x/w_qk/w_v/Q^T/K^T in
    bf16: every matmul weight becomes FWL-eligible and input DMA halves."""
    nsq = s // 512        # 512-wide s chunks
    nst = s // P          # 128-wide s tiles
    QB = min(1024, s)     # query-block width in phase C
    nqb = s // QB
    nc = bacc.Bacc("TRN2", target_bir_lowering=False, debug=False,
                   num_devices=NCORES)

    xT = nc.dram_tensor("xT", [E, s], qk_dt, kind="ExternalInput").ap()
    w_qk = nc.dram_tensor("w_qk", [E, HPC * 128], qk_dt, kind="ExternalInput").ap()
    b_qk = nc.dram_tensor("b_qk", [HPC * 128, 1], f32, kind="ExternalInput").ap()
    w_v = nc.dram_tensor("w_v", [E, HPC * D], qk_dt, kind="ExternalInput").ap()
    b_v = nc.dram_tensor("b_v", [P, HPC * D], f32, kind="ExternalInput").ap()
    w_out = nc.dram_tensor("w_out", [HPC * D, E], bf16, kind="ExternalInput").ap()
    outT = nc.dram_tensor("outT", [E, s], f32, kind="ExternalOutput").ap()
    scratch = nc.dram_tensor("scratch", [HPC * (s // min(1024, s)), min(1024, s)],
                             f32).ap()

    xT_r = xT.rearrange("(ko p) s -> p ko s", p=P)        # [128, 8, s]
    wqk_r = w_qk.rearrange("(ko p) f -> p ko f", p=P)     # [128, 8, 1024]
    wv_r = w_v.rearrange("(ko p) f -> p ko f", p=P)       # [128, 8, 512]
    bqk_r = b_qk.rearrange("(m p) one -> p (m one)", p=P)  # [128, 8]
    wo_r = w_out.rearrange("(j p) f -> p j f", p=P)       # [128, 4, 1024]
    outT_r = outT.rearrange("(m p) s -> p m s", p=P)      # [128, 8, s]

    with tile.TileContext(nc) as tc:
        def body():
            from contextlib import ExitStack
            with ExitStack() as outer:
                persist = outer.enter_context(tc.tile_pool(name="persist", bufs=1))
                qT2 = persist.tile([P, HPC // 2, s], qk_dt)
                kT2 = persist.tile([P, HPC // 2, s], qk_dt)
                v_sb = persist.tile([P, nst, HPC, 66], bf16)  # V + ones col
                attnT = persist.tile([P, HPC // 2, s], bf16)
                bqk_sb = persist.tile([P, HPC], f32)
                bv_sb = persist.tile([P, HPC, D], f32)
                wo_sb = persist.tile([P, 4, E], bf16)
                nc.sync.dma_start(bqk_sb[:], bqk_r)
                nc.sync.dma_start(bv_sb[:], b_v.rearrange("p (h d) -> p h d", d=D))
                nc.sync.dma_start(wo_sb[:], wo_r)
                nc.vector.memset(v_sb[:, :, :, 64:66], 1.0)

                # ---- Phase A/B: projections, streaming x^T in 512-col chunks
                with ExitStack() as ab:
                    xpool = ab.enter_context(tc.tile_pool(name="x", bufs=2))
                    wqk_pool = ab.enter_context(tc.tile_pool(name="wqk", bufs=1))
                    wv_pool = ab.enter_context(tc.tile_pool(name="wv", bufs=1))
                    psA = ab.enter_context(
                        tc.tile_pool(name="psA", bufs=3, space="PSUM"))
                    psB = ab.enter_context(
                        tc.tile_pool(name="psB", bufs=2, space="PSUM"))
                    wqk_sb = wqk_pool.tile([P, 8, HPC * 128], qk_dt)
                    wv_sb = wv_pool.tile([P, 8, HPC * D], qk_dt)
                    nc.sync.dma_start(wqk_sb[:], wqk_r)
                    nc.sync.dma_start(wv_sb[:], wv_r)

                    for q in range(nsq):
                        sq = slice(q * 512, (q + 1) * 512)
                        xt = xpool.tile([P, 8, 512], qk_dt)
                        nc.sync.dma_start(xt[:], xT_r[:, :, sq])
                        for m in range(HPC):  # qk feature tiles of 128
                            ps = psA.tile([P, 512], f32)
                            for k in range(8):
                                nc.tensor.matmul(
                                    ps[:], lhsT=wqk_sb[:, k, m * P:(m + 1) * P],
                                    rhs=xt[:, k, :],
                                    start=(k == 0), stop=(k == 7))
                            dst = qT2 if m % 2 == 0 else kT2
                            nc.vector.tensor_scalar_add(
                                dst[:, m // 2, sq], ps[:], bqk_sb[:, m:m + 1])
                        for stl in range(4):  # s tiles of 128 in this chunk
                            st = q * 4 + stl
                            ps = psB.tile([P, 512], f32)
                            for k in range(8):
                                nc.tensor.matmul(
                                    ps[:], lhsT=xt[:, k, stl * P:(stl + 1) * P],
                                    rhs=wv_sb[:, k, :],
                                    start=(k == 0), stop=(k == 7))
                            nc.vector.tensor_add(
                                v_sb[:, st, :, 0:D],
                                ps.rearrange("p (h d) -> p h d", d=D),
                                bv_sb[:])

                # ---- Phase C: attention per (head, q-block)
                with ExitStack() as c:
                    psS = c.enter_context(
                        tc.tile_pool(name="psS", bufs=2, space="PSUM"))
                    psAt = c.enter_context(
                        tc.tile_pool(name="psAt", bufs=2, space="PSUM"))
                    ppool = c.enter_context(tc.tile_pool(name="pT", bufs=3))
                    npool = c.enter_context(tc.tile_pool(name="norm", bufs=2))
                    bpool = c.enter_context(tc.tile_pool(name="bc", bufs=2))
                    spool = c.enter_context(tc.tile_pool(name="asb", bufs=2))
                    for i in range(HPC):
                        u, poff = i // 2, (i % 2) * 64
                        QT = qT2[poff:poff + 64, u, :]
                        KT = kT2[poff:poff + 64, u, :]
                        for qb in range(nqb):
                            at = psAt.tile([66, QB], f32,
                                           name=f"at{i}_{qb}", tag="at")

                            def emit_pv(skt, pT):
                                for q2 in range(QB // 512):
                                    nc.tensor.matmul(
                                        at[:, q2 * 512:(q2 + 1) * 512],
                                        lhsT=v_sb[:, skt, i, :],
                                        rhs=pT[:, q2 * 512:(q2 + 1) * 512],
                                        start=(skt == 0),
                                        stop=(skt == nst - 1))

                            prev = None
                            for skt in range(nst):
                                ps_s = psS.tile([P, QB], f32,
                                                name="ps_s", tag="ps_s")
                                for q2 in range(QB // 512):
                                    nc.tensor.matmul(
                                        ps_s[:, q2 * 512:(q2 + 1) * 512],
                                        lhsT=KT[:, skt * P:(skt + 1) * P],
                                        rhs=QT[:, qb * QB + q2 * 512:
                                               qb * QB + (q2 + 1) * 512],
                                        start=True, stop=True)
                                pT = ppool.tile([P, QB], bf16,
                                                name="pT", tag="pT")
                                nc.scalar.activation(pT[:], ps_s[:], AF.Exp,
                                                     scale=SCALE)
                                if prev is not None:
                                    emit_pv(*prev)
                                prev = (skt, pT)
                            emit_pv(*prev)

                            # normalize: recip of denominators (row D), gpsimd
                            # partition-broadcast, multiply into attnT
                            attn_sb = spool.tile([66, QB], f32, tag="asb")
                            nc.vector.tensor_copy(attn_sb[:], at[:])
                            # broadcast the denominator row first, then recip
                            # on 64 partitions (single-partition custom-DVE
                            # ops are broken on HW)
                            si = i * nqb + qb
                            nc.sync.dma_start(scratch[si:si + 1, :],
                                              attn_sb[D:D + 1, :])
                            bc = bpool.tile([64, QB], f32, tag="bc")
                            nc.sync.dma_start(
                                bc[:], scratch[si:si + 1, :]
                                .partition_broadcast(64)
                                .rearrange("p one s -> p (one s)"))
                            rb = npool.tile([64, QB], f32, tag="recip")
                            nc.vector.reciprocal_approx_fast(rb[:], bc[:])
                            nc.vector.tensor_mul(
                                attnT[poff:poff + 64, u,
                                      qb * QB:(qb + 1) * QB],
                                attn_sb[0:D, :], rb[:])

                # ---- Phase D: output projection (partial; host sums pairs)
                with ExitStack() as d:
                    psD = d.enter_context(
                        tc.tile_pool(name="psD", bufs=2, space="PSUM"))
                    opool = d.enter_context(tc.tile_pool(name="osb", bufs=2))
                    for m in range(8):
                        ps_o = psD.tile([P, s], f32)
                        for k in range(4):
                            for n4 in range(nsq):
                                nc.tensor.matmul(
                                    ps_o[:, n4 * 512:(n4 + 1) * 512],
                                    lhsT=wo_sb[:, k, m * P:(m + 1) * P],
                                    rhs=attnT[:, k, n4 * 512:(n4 + 1) * 512],
                                    start=(k == 0), stop=(k == 3))
                        o_sb = opool.tile([P, s], f32)
                        nc.vector.tensor_copy(o_sb[:], ps_o[:])
                        nc.sync.dma_start(outT_r[:, m, :], o_sb[:])

        if repeat > 1:
            with tc.For_i(0, repeat, 1):
                body()
        else:
            body()

    nc.compile()
    return nc


def build_nc_v4(s=S, repeat=1, qk_dt=f32r):
    """v4 = v2 + pair-projection import into phase C + DMA reordering +
    direct PSUM->DRAM output.

    Upfront (A0): V projection for all heads + QK projection for pair 0 only.
    Phase C: while heads of pair u run attention (ACT-bound), the QK
    projection matmuls for pair u+1 are interleaved one per skt iteration,
    using 2 spare PSUM banks and re-streamed x chunks. Phase D DMAs PSUM
    accumulators straight to DRAM."""
    nsq = s // 512        # 512-wide s chunks
    nst = s // P          # 128-wide s tiles
    QB = min(1024, s)     # query-block width in phase C
    nqb = s // QB
    npair = HPC // 2
    nc = bacc.Bacc("TRN2", target_bir_lowering=False, debug=False,
                   num_devices=NCORES)

    xT = nc.dram_tensor("xT", [E, s], qk_dt, kind="ExternalInput").ap()
    w_qk = nc.dram_tensor("w_qk", [E, HPC * 128], qk_dt, kind="ExternalInput").ap()
    b_qk = nc.dram_tensor("b_qk", [HPC * 128, 1], f32, kind="ExternalInput").ap()
    w_v = nc.dram_tensor("w_v", [E, HPC * D], qk_dt, kind="ExternalInput").ap()
    b_v = nc.dram_tensor("b_v", [P, HPC * D], f32, kind="ExternalInput").ap()
    w_out = nc.dram_tensor("w_out", [HPC * D, E], bf16, kind="ExternalInput").ap()
    outT = nc.dram_tensor("outT", [E, s], bf16, kind="ExternalOutput").ap()
    scratch = nc.dram_tensor("scratch", [HPC * (s // min(1024, s)), min(1024, s)],
                             f32).ap()

    xT_r = xT.rearrange("(ko p) s -> p ko s", p=P)        # [128, 8, s]
    wqk_r = w_qk.rearrange("(ko p) f -> p ko f", p=P)     # [128, 8, 1024]
    wv_r = w_v.rearrange("(ko p) f -> p ko f", p=P)     # [128, 8, 512]
    bqk_r = b_qk.rearrange("(m p) one -> p (m one)", p=P)  # [128, 8]
    wo_r = w_out.rearrange("(j p) f -> p j f", p=P)       # [128, 4, 1024]
    outT_r = outT.rearrange("(m p) s -> p m s", p=P)      # [128, 8, s]

    with tile.TileContext(nc) as tc:
        def body():
            from contextlib import ExitStack
            with ExitStack() as outer:
                persist = outer.enter_context(tc.tile_pool(name="persist", bufs=1))
                qT2 = persist.tile([P, npair, s], qk_dt)
                kT2 = persist.tile([P, npair, s], qk_dt)
                v_sb = persist.tile([P, nst, HPC, 66], bf16)
                attnT = persist.tile([P, npair, s], bf16)
                bqk_sb = persist.tile([P, HPC], f32)
                bv_sb = persist.tile([P, HPC, D], f32)
                wo_sb = persist.tile([P, 4, E], bf16)
                o_part = persist.tile([P, 8, s], bf16)
                wqk_pool = outer.enter_context(tc.tile_pool(name="wqk", bufs=1))
                wqk_sb = wqk_pool.tile([P, 8, HPC * 128], qk_dt)
                xpool = outer.enter_context(tc.tile_pool(name="x", bufs=2))

                def qk_mm(pool, pair, chunk, qk, xt):
                    """One m-group (8 accumulating MMs + bias add) of the QK
                    projection for `pair`; qk=0 -> q cols, 1 -> k cols."""
                    m = 2 * pair + qk
                    sq = slice(chunk * 512, (chunk + 1) * 512)
                    ps = pool.tile([P, 512], f32, name="psProj", tag="psProj")
                    for k in range(8):
                        nc.tensor.matmul(
                            ps[:], lhsT=wqk_sb[:, k, m * P:(m + 1) * P],
                            rhs=xt[:, k, :],
                            start=(k == 0), stop=(k == 7))
                    dst = qT2 if qk == 0 else kT2
                    nc.vector.tensor_scalar_add(
                        dst[:, pair, sq], ps[:], bqk_sb[:, m:m + 1])

                # ---- A0: stream x; V proj (all heads) + QK proj pair 0
                with ExitStack() as ab:
                    wv_pool = ab.enter_context(tc.tile_pool(name="wv", bufs=1))
                    psProj = ab.enter_context(
                        tc.tile_pool(name="psA", bufs=3, space="PSUM"))
                    psB = ab.enter_context(
                        tc.tile_pool(name="psB", bufs=2, space="PSUM"))
                    wv_sb = wv_pool.tile([P, 8, HPC * D], qk_dt)
                    # DMA order: x chunk 0 + first wqk half unblock compute
                    # fastest; wv next; wo (phase D) last.
                    xts = []
                    for q in range(nsq):
                        xt = xpool.tile([P, 8, 512], qk_dt,
                                        name=f"xt{q}", tag="xt")
                        if q == 0:
                            nc.sync.dma_start(xt[:], xT_r[:, :, 0:512])
                            nc.sync.dma_start(bqk_sb[:], bqk_r)
                            nc.sync.dma_start(
                                wqk_sb[:, :, 0:512], wqk_r[:, :, 0:512])
                            nc.sync.dma_start(
                                wqk_sb[:, :, 512:1024], wqk_r[:, :, 512:1024])
                            nc.sync.dma_start(wv_sb[:], wv_r)
                            nc.sync.dma_start(
                                bv_sb[:], b_v.rearrange("p (h d) -> p h d", d=D))
                            nc.vector.memset(
                                v_sb[:, :, :, 64:66], 1.0)
                        else:
                            sq = slice(q * 512, (q + 1) * 512)
                            nc.sync.dma_start(xt[:], xT_r[:, :, sq])
                        xts.append(xt)

                    imports_on = (nqb == 2 and nst == 16)
                    pairs_up = [0] if imports_on else list(range(npair))
                    for q in range(nsq):
                        xt = xts[q]
                        for pu in pairs_up:
                            qk_mm(psProj, pu, q, 0, xt)
                            qk_mm(psProj, pu, q, 1, xt)
                        for stl in range(4):  # V proj s tiles of 128
                            st = q * 4 + stl
                            ps = psB.tile([P, 512], f32)
                            for k in range(8):
                                nc.tensor.matmul(
                                    ps[:], lhsT=xt[:, k, stl * P:(stl + 1) * P],
                                    rhs=wv_sb[:, k, :],
                                    start=(k == 0), stop=(k == 7))
                            nc.vector.tensor_add(
                                v_sb[:, st, :, 0:D],
                                ps.rearrange("p (h d) -> p h d", d=D),
                                bv_sb[:])
                    nc.sync.dma_start(wo_sb[:], wo_r)

                # ---- Phase C with imported pair-projections
                with ExitStack() as c:
                    psS = c.enter_context(
                        tc.tile_pool(name="psS", bufs=2, space="PSUM"))
                    psAt = c.enter_context(
                        tc.tile_pool(name="psAt", bufs=1, space="PSUM"))
                    psProj = c.enter_context(
                        tc.tile_pool(name="psProj", bufs=2, space="PSUM"))
                    ppool = c.enter_context(tc.tile_pool(name="pT", bufs=3))
                    npool = c.enter_context(tc.tile_pool(name="norm", bufs=2))
                    bpool = c.enter_context(tc.tile_pool(name="bc", bufs=2))
                    spool = c.enter_context(tc.tile_pool(name="asb", bufs=2))
                    for i in range(HPC):
                        u, poff = i // 2, (i % 2) * 64
                        QT = qT2[poff:poff + 64, u, :]
                        KT = kT2[poff:poff + 64, u, :]
                        # sections: section idx within pair = (i%2)*nqb + qb.
                        # During pair u's 2*nqb sections, import pair u+1's
                        # QK projection (2 m-groups x nsq chunks).
                        for qb in range(nqb):
                            sec = (i % 2) * nqb + qb
                            do_import = (u + 1 < npair and nqb == 2
                                         and nst == 16 and imports_on)
                            do_dimp = (u == npair - 1 and nqb == 2
                                       and nst == 16 and imports_on)
                            at = psAt.tile([66, QB], f32,
                                           name=f"at{i}_{qb}", tag="at")

                            def emit_pv(skt, pT):
                                for q2 in range(QB // 512):
                                    nc.tensor.matmul(
                                        at[:, q2 * 512:(q2 + 1) * 512],
                                        lhsT=v_sb[:, skt, i, :],
                                        rhs=pT[:, q2 * 512:(q2 + 1) * 512],
                                        start=(skt == 0),
                                        stop=(skt == nst - 1))

                            if do_import:
                                xt_imp = xpool.tile([P, 8, 512], qk_dt,
                                                    name=f"xti{i}{qb}",
                                                    tag="xt")
                                sq = slice(sec * 512, (sec + 1) * 512)
                                nc.sync.dma_start(xt_imp[:], xT_r[:, :, sq])
                            prev = None
                            ps_imp = None
                            for skt in range(nst):
                                ps_s = psS.tile([P, QB], f32,
                                                name="ps_s", tag="ps_s")
                                for q2 in range(QB // 512):
                                    nc.tensor.matmul(
                                        ps_s[:, q2 * 512:(q2 + 1) * 512],
                                        lhsT=KT[:, skt * P:(skt + 1) * P],
                                        rhs=QT[:, qb * QB + q2 * 512:
                                               qb * QB + (q2 + 1) * 512],
                                        start=True, stop=True)
                                if do_import:
                                    # one imported projection MM per skt:
                                    # qk m-group 0 over skt 0-7, 1 over 8-15
                                    qk_, k_ = skt // 8, skt % 8
                                    m_ = 2 * (u + 1) + qk_
                                    if k_ == 0:
                                        ps_imp = psProj.tile(
                                            [P, 512], f32, name="psProj",
                                            tag="psProj")
                                    nc.tensor.matmul(
                                        ps_imp[:],
                                        lhsT=wqk_sb[:, k_, m_ * P:(m_ + 1) * P],
                                        rhs=xt_imp[:, k_, :],
                                        start=(k_ == 0), stop=(k_ == 7))
                                    if k_ == 7:
                                        dst = qT2 if qk_ == 0 else kT2
                                        nc.vector.tensor_scalar_add(
                                            dst[:, u + 1,
                                                sec * 512:(sec + 1) * 512],
                                            ps_imp[:], bqk_sb[:, m_:m_ + 1])
                                if do_dimp:
                                    # pair 3 has no projections left: import
                                    # phase-D k=0,1 partials (pairs 0,1 are
                                    # done), one MM per skt slot
                                    t = sec * nst + skt
                                    g, k_ = t // 2, t % 2
                                    m_, n4_ = g // nsq, g % nsq
                                    if k_ == 0:
                                        ps_imp = psProj.tile(
                                            [P, 512], f32, name="psProj",
                                            tag="psProj")
                                    nc.tensor.matmul(
                                        ps_imp[:],
                                        lhsT=wo_sb[:, k_, m_ * P:(m_ + 1) * P],
                                        rhs=attnT[:, k_,
                                                  n4_ * 512:(n4_ + 1) * 512],
                                        start=(k_ == 0), stop=(k_ == 1))
                                    if k_ == 1:
                                        nc.vector.tensor_copy(
                                            o_part[:, m_,
                                                   n4_ * 512:(n4_ + 1) * 512],
                                            ps_imp[:])
                                pT = ppool.tile([P, QB], bf16,
                                                name="pT", tag="pT")
                                nc.scalar.activation(pT[:], ps_s[:], AF.Exp,
                                                     scale=SCALE)
                                if prev is not None:
                                    emit_pv(*prev)
                                prev = (skt, pT)
                            emit_pv(*prev)

                            attn_sb = spool.tile([66, QB], f32, tag="asb")
                            nc.vector.tensor_copy(attn_sb[:], at[:])
                            # broadcast the denominator row first, then recip
                            # on 64 partitions (single-partition custom-DVE
                            # ops are broken on HW)
                            si = i * nqb + qb
                            nc.gpsimd.dma_start(scratch[si:si + 1, :],
                                                attn_sb[D:D + 1, :])
                            bc = bpool.tile([64, QB], f32, tag="bc")
                            nc.gpsimd.dma_start(
                                bc[:], scratch[si:si + 1, :]
                                .partition_broadcast(64)
                                .rearrange("p one s -> p (one s)"))
                            rb = npool.tile([64, QB], f32, tag="recip")
                            nc.vector.reciprocal_approx_fast(rb[:], bc[:])
                            nc.vector.tensor_mul(
                                attnT[poff:poff + 64, u,
                                      qb * QB:(qb + 1) * QB],
                                attn_sb[0:D, :], rb[:])

                # ---- Phase D: output projection, PSUM -> DRAM directly
                with ExitStack() as d:
                    psD = d.enter_context(
                        tc.tile_pool(name="psD", bufs=2, space="PSUM"))
                    opool = d.enter_context(tc.tile_pool(name="osb", bufs=2))
                    dks = (2, 3) if imports_on else (0, 1, 2, 3)
                    for m in range(8):
                        ps_o = psD.tile([P, s], f32)
                        for j, k in enumerate(dks):
                            for n4 in range(nsq):
                                nc.tensor.matmul(
                                    ps_o[:, n4 * 512:(n4 + 1) * 512],
                                    lhsT=wo_sb[:, k, m * P:(m + 1) * P],
                                    rhs=attnT[:, k, n4 * 512:(n4 + 1) * 512],
                                    start=(j == 0), stop=(j == len(dks) - 1))
                        o_sb = opool.tile([P, s], bf16)
                        if imports_on:
                            nc.vector.tensor_add(o_sb[:], ps_o[:],
                                                 o_part[:, m, :])
                        else:
                            nc.vector.tensor_copy(o_sb[:], ps_o[:])
                        nc.gpsimd.dma_start(outT_r[:, m, :], o_sb[:])

        if repeat > 1:
            with tc.For_i(0, repeat, 1):
                body()
        else:
            body()

    nc.compile()
    return nc


VARIANT = os.environ.get("K_VARIANT", "v5")
HALF_SCORES = os.environ.get("K_HALF_SCORES", "0") == "1"


def _get_nc(s=S, repeat=1, variant=None):
    if variant is None:
        variant = VARIANT
    key = (s, repeat, variant)
    if key not in _BUILD_CACHE:
        if variant == "v2":
            _BUILD_CACHE[key] = build_nc_v2(s=s, repeat=repeat)
        elif variant == "v3":
            _BUILD_CACHE[key] = build_nc_v2(s=s, repeat=repeat, qk_dt=bf16)
        elif variant == "v4":
            _BUILD_CACHE[key] = build_nc_v4(s=s, repeat=repeat)
        elif variant == "v5":
            _BUILD_CACHE[key] = build_nc_v4(s=s, repeat=repeat, qk_dt=bf16)
        else:
            _BUILD_CACHE[key] = build_nc(s=s, repeat=repeat,
                                         half_scores=(variant == "v1h" or
                                                      HALF_SCORES))
    return _BUILD_CACHE[key]


def shard_inputs(x, w_qkv, b_qkv, w_out, b_out):
    """Host-side sharding: per-core input maps."""
    in_maps = []
    for c in range(NCORES):
        b, g = c // 2, c % 2
        heads = [g * HPC + i for i in range(HPC)]
        # qk columns, pair-interleaved: block 2u = q cols of head pair u,
        # block 2u+1 = k cols of head pair u
        qk_cols, qk_bias = [], []
        for u in range(HPC // 2):
            hA, hB = heads[2 * u], heads[2 * u + 1]
            for off in (0, 64):  # 0: q, 64: k
                for h in (hA, hB):
                    qk_cols.append(w_qkv[:, h * 192 + off:h * 192 + off + 64])
                    qk_bias.append(b_qkv[h * 192 + off:h * 192 + off + 64])
        w_qk_c = np.ascontiguousarray(np.concatenate(qk_cols, axis=1))
        b_qk_c = np.ascontiguousarray(
            np.concatenate(qk_bias)[:, None].astype(np.float32))
        w_v_c = np.ascontiguousarray(np.concatenate(
            [w_qkv[:, h * 192 + 128:h * 192 + 192] for h in heads], axis=1))
        b_v_c = np.ascontiguousarray(np.broadcast_to(np.concatenate(
            [b_qkv[h * 192 + 128:h * 192 + 192] for h in heads])[None, :],
            (P, HPC * D)).astype(np.float32))
        w_out_c = np.ascontiguousarray(np.concatenate(
            [w_out[h * D:(h + 1) * D, :] for h in heads], axis=0))
        xT_c = np.ascontiguousarray(x[b].T)
        if VARIANT in ("v2", "v3", "v4", "v5"):
            import ml_dtypes
            w_out_c = w_out_c.astype(ml_dtypes.bfloat16)
            if VARIANT in ("v3", "v5"):
                xT_c = xT_c.astype(ml_dtypes.bfloat16)
                w_qk_c = w_qk_c.astype(ml_dtypes.bfloat16)
                w_v_c = w_v_c.astype(ml_dtypes.bfloat16)
        in_maps.append({
            "xT": xT_c, "w_qk": w_qk_c, "b_qk": b_qk_c,
            "w_v": w_v_c, "b_v": b_v_c, "w_out": w_out_c,
        })
    return in_maps


def unshard_output(results, b_out):
    out = np.empty((B, S, E), dtype=np.float32)
    for b in range(B):
        acc = (np.asarray(results[2 * b]["outT"], dtype=np.float32) +
               np.asarray(results[2 * b + 1]["outT"], dtype=np.float32))
        out[b] = acc.T + b_out
    return out


def kernel(x, w_qkv, b_qkv, w_out, b_out):
    global LAST_RESULTS
    x = np.asarray(x, dtype=np.float32)
    w_qkv = np.asarray(w_qkv, dtype=np.float32)
    b_qkv = np.asarray(b_qkv, dtype=np.float32)
    w_out = np.asarray(w_out, dtype=np.float32)
    b_out = np.asarray(b_out, dtype=np.float32)

    nc = _get_nc()
    in_maps = shard_inputs(x, w_qkv, b_qkv, w_out, b_out)
    try:
        res = run_bass_kernel_spmd(nc, in_maps, list(range(NCORES)))
    except ModuleNotFoundError:
        # BASS_TRACE requested but this axon client has no NTFF hook module
        os.environ["BASS_NEVER_TRACE"] = "1"
        res = run_bass_kernel_spmd(nc, in_maps, list(range(NCORES)))
    LAST_RESULTS = res
    return unshard_output(res.results, b_out)

